# revision 3
# baseline (speedup 1.0000x reference)
"""CrossViewTransformer kernel for 8 Trainium2 NeuronCores.

Problem: B=4, C=256, H=W=64 (N=4096), Cqk=32 cross-attention + residual.
  Q = Wq@src, K = Wk@tgt, V = Wv@tgt  (1x1 convs over channels)
  out = softmax(Q^T K) @ V^T + src     (no 1/sqrt(d) scale)

Sharding: 8 cores = 4 batches x 2 query-halves. Each core computes attention
for 2048 queries x 4096 keys of one batch.

Per-core pipeline, per 512-query chunk (32 m-tiles of 128 keys):
  - scoresT m-tiles are single-PSUM-bank [128,512] tiles in a 4-deep ring,
    QK matmuls run 2 m-tiles ahead of consumption (row-tile packed 2-way).
  - The softmax exp is split across TWO engines: ACT computes true exp
    (scale=1/A folded in) into fp8e4m3; DVE computes a Schraudolph-style
    exp for its share of m-tiles in ONE tensor_scalar op: the Q projection
    is pre-scaled by A = 8*log2(e), so bits = trunc(max(S' + B, 0)) is the
    fp8 bit pattern of ~exp(s), written as uint8 and bitcast to fp8.
  - attn@V: fp8 DoubleRow matmuls (2 m-tiles per instruction) accumulate
    [c_half, q] in PSUM across all 32 m-tiles.
  - denominator: end-of-chunk burst of 32 plain fp8 ones-matmuls packed
    4-way into PE column-tile positions, accumulating rows {0,32,64,96}
    of one PSUM bank; folded by gpsimd partition_all_reduce, approximate
    reciprocal on DVE, out = av * r + src_res -> DRAM.
"""

import os
import sys

sys.path.insert(0, "/opt/trn_rl_repo")

import numpy as np
import ml_dtypes

BF16 = ml_dtypes.bfloat16

B, C, H, W = 4, 256, 64, 64
N = H * W            # 4096 keys (and queries per batch)
CQK = 32
NCORES = 8
QSH = N // 2         # 2048 queries per core
QC = 512             # q-chunk width (one PSUM bank)
NQC = QSH // QC      # 4 q-chunks
MT = 128             # m-tile (keys per scoresT tile)
NMT = N // MT        # 32 m-tiles
NPAIR = NMT // 2     # 16 m-tile pairs per chunk

# Schraudolph constants: scores arrive pre-scaled by A (folded into Wq);
# fp8e4m3 bits of ~exp(s) = trunc(max(A*s + BEXP, 0)).
AEXP = 8.0 * 1.4426950408889634
BEXP = float(os.environ.get("KERNEL_BEXP", "56.12"))

SBUFS = int(os.environ.get("KERNEL_SBUFS", "4"))     # scoresT ring depth
ACT_TILES = int(os.environ.get("KERNEL_ACT_TILES", "18"))  # of 32 per chunk
LOOP = int(os.environ.get("KERNEL_LOOP", "0"))  # >0: repeat body for timing
# timing bisection: 0=empty loop body, 1=+proj, 2=+QK, 3=+exp, 4=+AV,
# 5=+L matmuls, 6=full
STAGE = int(os.environ.get("KERNEL_STAGE", "6"))
QKAHEAD = int(os.environ.get("KERNEL_QKAHEAD", "3"))  # QK lookahead (<SBUFS)

_last_results = None  # BassKernelResults of the most recent run (for test.py)


def _act_mask():
    """Spread ACT_TILES of the 32 m-tiles evenly onto ACT; rest on DVE."""
    mask = [False] * NMT
    acc = 0
    for g in range(NMT):
        acc += ACT_TILES
        if acc >= NMT:
            acc -= NMT
            mask[g] = True
    return mask


def _build_bass():
    import concourse.bass as bass
    import concourse.tile as tile
    from concourse import bacc, mybir
    from concourse import bass_isa
    from contextlib import ExitStack

    f32 = mybir.dt.float32
    bf16 = mybir.dt.bfloat16
    fp8 = mybir.dt.float8e4
    u8 = mybir.dt.uint8
    AT_NP = ml_dtypes.float8_e4m3

    nc = bacc.Bacc("TRN2")

    # ---- DRAM I/O ----
    tgt_d = nc.dram_tensor("tgt", [C, N], bf16, kind="ExternalInput")
    srcq_d = nc.dram_tensor("srcq", [C, QSH], bf16, kind="ExternalInput")
    srcr_d = nc.dram_tensor("srcr", [C, QSH], f32, kind="ExternalInput")
    wqT_d = nc.dram_tensor("wqT", [C, CQK], bf16, kind="ExternalInput")
    wkT_d = nc.dram_tensor("wkT", [C, CQK], bf16, kind="ExternalInput")
    wvT_d = nc.dram_tensor("wvT", [C, C], bf16, kind="ExternalInput")
    bqk_d = nc.dram_tensor("bqk", [128, 2], f32, kind="ExternalInput")
    bv_d = nc.dram_tensor("bv", [1, C], bf16, kind="ExternalInput")
    out_d = nc.dram_tensor("out", [C, QSH], f32, kind="ExternalOutput")

    ones_col_d = nc.inline_tensor(np.ones((128, 1), dtype=AT_NP), name="ones_col")

    act_mask = _act_mask()

    with tile.TileContext(nc) as tc:
        with (
            tc.tile_pool(name="const", bufs=1) as const,
            tc.tile_pool(name="data", bufs=1) as data,
        ):
            # ---- ACT table warmup: a dependency-free Exp so walrus's
            # inserted ACT_TABLE_LOAD lands on an instruction with <=1 wait.
            warm = const.tile([1, 8], f32, tag="warm")
            nc.vector.memset(warm, 0.0)
            nc.scalar.activation(
                out=warm, in_=warm, func=mybir.ActivationFunctionType.Exp
            )

            # ---- constants / weights ----
            wq0 = const.tile([128, CQK], bf16, tag="wq0")
            wq1 = const.tile([128, CQK], bf16, tag="wq1")
            wk0 = const.tile([128, CQK], bf16, tag="wk0")
            wk1 = const.tile([128, CQK], bf16, tag="wk1")
            wv0 = const.tile([128, C], bf16, tag="wv0")
            wv1 = const.tile([128, C], bf16, tag="wv1")
            bqk = const.tile([128, 2], f32, tag="bqk")
            bvr = const.tile([1, C], bf16, tag="bvr")
            ones_col = const.tile([128, 1], fp8, tag="ones_col")
            nc.sync.dma_start(out=wq0, in_=wqT_d[0:128, :])
            nc.sync.dma_start(out=wq1, in_=wqT_d[128:256, :])
            nc.sync.dma_start(out=wk0, in_=wkT_d[0:128, :])
            nc.sync.dma_start(out=wk1, in_=wkT_d[128:256, :])
            nc.sync.dma_start(out=wv0, in_=wvT_d[0:128, :])
            nc.sync.dma_start(out=wv1, in_=wvT_d[128:256, :])
            nc.sync.dma_start(out=bqk, in_=bqk_d[:, :])
            nc.sync.dma_start(out=bvr, in_=bv_d[:, :])
            nc.sync.dma_start(out=ones_col, in_=ones_col_d[:, :])

            # ---- big data tiles ----
            tgt0 = data.tile([128, 8, 512], bf16, tag="tgt0")
            tgt1 = data.tile([128, 8, 512], bf16, tag="tgt1")
            for j in range(8):
                sl = slice(j * 512, (j + 1) * 512)
                nc.sync.dma_start(out=tgt0[:, j, :], in_=tgt_d[0:128, sl])
                nc.sync.dma_start(out=tgt1[:, j, :], in_=tgt_d[128:256, sl])
            srcq0 = data.tile([128, NQC, QC], bf16, tag="srcq0")
            srcq1 = data.tile([128, NQC, QC], bf16, tag="srcq1")
            srcr0 = data.tile([128, NQC, QC], f32, tag="srcr0")
            srcr1 = data.tile([128, NQC, QC], f32, tag="srcr1")
            for j in range(NQC):
                sl = slice(j * QC, (j + 1) * QC)
                nc.sync.dma_start(out=srcq0[:, j, :], in_=srcq_d[0:128, sl])
                nc.sync.dma_start(out=srcq1[:, j, :], in_=srcq_d[128:256, sl])
                nc.sync.dma_start(out=srcr0[:, j, :], in_=srcr_d[0:128, sl])
                nc.sync.dma_start(out=srcr1[:, j, :], in_=srcr_d[128:256, sl])

            # bv broadcast to all partitions once (for the VT copy+bias add)
            bv_rep = data.tile([128, C], bf16, tag="bv_rep")
            nc.gpsimd.partition_broadcast(bv_rep, bvr)

            # K4_sb[32*i + cqk, g, col] = K[cqk, (4g+i)*128 + col]
            # (4 m-tiles of a group live at partition blocks 0/32/64/96)
            K4_sb = data.tile([128, N // (4 * MT), MT], bf16, tag="K4_sb")
            # Q replicated at all 4 partition blocks
            Q4_sb = data.tile([128, NQC, QC], bf16, tag="Q4_sb")
            VT_sb = data.tile([128, NMT, C], fp8, tag="VT_sb")

            # staging tile for the denominator fold (rows 0/32/64/96 carry
            # the 4 col-group partials, the rest stay zero)
            lrow_sb = data.tile([128, QC], f32, tag="lrow_sb")
            nc.vector.memset(lrow_sb, 0.0)

            body_stack = ExitStack()
            if LOOP:
                body_stack.enter_context(tc.For_i(0, LOOP, 1))
            with body_stack:
                if STAGE == 0:
                    tick = data.tile([1, 8], f32, tag="tick")
                    nc.vector.memset(tick, 1.0)
                # ---- projections ----
                if STAGE >= 1:
                    with (
                        tc.tile_pool(name="pv", bufs=3, space="PSUM") as pv,
                        tc.tile_pool(name="pk", bufs=2, space="PSUM") as pk,
                    ):
                        # V^T tiles: VT[m,c] = sum_ch tgt[ch,m] WvT[ch,c] (+ bv)
                        for mt in range(NMT):
                            ps = pv.tile([128, C], f32, tag="psv")
                            j, o = divmod(mt * MT, 512)
                            lhs0 = tgt0[:, j, o : o + MT]
                            lhs1 = tgt1[:, j, o : o + MT]
                            nc.tensor.matmul(
                                ps, lhsT=lhs0, rhs=wv0, start=True, stop=False
                            )
                            nc.tensor.matmul(
                                ps, lhsT=lhs1, rhs=wv1, start=False, stop=True
                            )
                            nc.vector.tensor_add(VT_sb[:, mt, :], ps, bv_rep)
                        # K: 4 m-tiles per group at partition blocks (col groups)
                        for g in range(N // (4 * MT)):
                            ps = pk.tile([128, MT], f32, tag="psk")
                            for i in range(4):
                                mt = 4 * g + i
                                j, o = divmod(mt * MT, 512)
                                out_blk = ps[32 * i : 32 * (i + 1), :]
                                nc.tensor.matmul(
                                    out_blk, lhsT=wk0, rhs=tgt0[:, j, o : o + MT],
                                    start=True, stop=False, tile_position=(0, 32 * i),
                                )
                                nc.tensor.matmul(
                                    out_blk, lhsT=wk1, rhs=tgt1[:, j, o : o + MT],
                                    start=False, stop=True, tile_position=(0, 32 * i),
                                )
                            nc.vector.tensor_scalar_add(K4_sb[:, g, :], ps, bqk[:, 1:2])
                        # Q: proj into partition block 0, then replicate via DMA
                        for j in range(NQC):
                            ps = pk.tile([CQK, QC], f32, tag="psq")
                            nc.tensor.matmul(
                                ps, lhsT=wq0, rhs=srcq0[:, j, :], start=True, stop=False
                            )
                            nc.tensor.matmul(
                                ps, lhsT=wq1, rhs=srcq1[:, j, :], start=False, stop=True
                            )
                            nc.vector.tensor_scalar_add(
                                Q4_sb[0:CQK, j, :], ps, bqk[0:CQK, 0:1]
                            )
                        for i in range(1, 4):
                            nc.sync.dma_start(
                                out=Q4_sb[32 * i : 32 * (i + 1), :, :],
                                in_=Q4_sb[0:CQK, :, :],
                            )

                # ---- attention ----
                if STAGE >= 2:
                    with (
                        tc.tile_pool(name="ps_s", bufs=SBUFS, space="PSUM") as ps_s,
                        tc.tile_pool(name="ps_av", bufs=1, space="PSUM") as ps_av,
                        tc.tile_pool(name="ps_l", bufs=1, space="PSUM") as ps_l,
                        tc.tile_pool(name="att", bufs=NPAIR) as att,
                        tc.tile_pool(name="outp", bufs=4) as outp,
                    ):
                        # one PSUM bank for the denominator rows; rows other
                        # than {0,32,64,96} must read as 0 for the fold, so
                        # zero the whole bank once per pass
                        lrow = ps_l.tile([128, QC], f32, tag="lrow")
                        nc.vector.memset(lrow, 0.0)
                        for qc in range(NQC):
                            av0 = ps_av.tile([128, QC], f32, tag="av0")
                            av1 = ps_av.tile([128, QC], f32, tag="av1")

                            s_ring = [None] * NMT

                            def emit_qk(g):
                                S = ps_s.tile([128, QC], f32, tag="S")
                                gg, ii = divmod(g, 4)
                                blk = slice(32 * ii, 32 * (ii + 1))
                                nc.tensor.matmul(
                                    S,
                                    lhsT=K4_sb[blk, gg, :],
                                    rhs=Q4_sb[blk, qc, :],
                                    start=True,
                                    stop=True,
                                    tile_position=(32 * ii, 0),
                                )
                                s_ring[g] = S

                            for g in range(QKAHEAD):
                                emit_qk(g)

                            pairs = [None] * NPAIR
                            for p in range(NPAIR):
                                g0 = 2 * p
                                pair = att.tile([128, 2, QC], fp8, tag="pair")
                                pairs[p] = pair
                                if STAGE >= 3:
                                    for j in (0, 1):
                                        g = g0 + j
                                        S = s_ring[g]
                                        if act_mask[g]:
                                            nc.scalar.activation(
                                                out=pair[:, j, :],
                                                in_=S,
                                                func=mybir.ActivationFunctionType.Exp,
                                                scale=1.0 / AEXP,
                                            )
                                        else:
                                            nc.vector.tensor_scalar(
                                                out=pair[:, j, :].bitcast(u8),
                                                in0=S,
                                                scalar1=BEXP,
                                                scalar2=0.0,
                                                op0=mybir.AluOpType.add,
                                                op1=mybir.AluOpType.max,
                                            )
                                for gq in (2 * p + QKAHEAD, 2 * p + QKAHEAD + 1):
                                    if gq < NMT:
                                        emit_qk(gq)
                                if STAGE >= 4:
                                    for av, cs in (
                                        (av0, slice(0, 128)),
                                        (av1, slice(128, 256)),
                                    ):
                                        nc.tensor.matmul(
                                            av,
                                            lhsT=VT_sb[:, g0 : g0 + 2, cs],
                                            rhs=pair[:, 0:2, :],
                                            start=p == 0,
                                            stop=p == NPAIR - 1,
                                            perf_mode=mybir.MatmulPerfMode.DoubleRow,
                                        )
                            if STAGE < 4:
                                continue
                            # free the av banks fast: PSUM->SBUF copies on ACT
                            av0_sb = outp.tile([128, QC], f32, tag="av0_sb")
                            av1_sb = outp.tile([128, QC], f32, tag="av1_sb")
                            nc.scalar.copy(out=av0_sb, in_=av0)
                            nc.scalar.copy(out=av1_sb, in_=av1)
                            if STAGE < 5:
                                continue
                            # denominator burst: 32 ones-matmuls, 4-way
                            # column-tile packed, accumulating rows {0,32,64,96}
                            for g in range(NMT):
                                jj = g % 4
                                nc.tensor.matmul(
                                    lrow[32 * jj : 32 * jj + 1, :],
                                    lhsT=ones_col,
                                    rhs=pairs[g // 2][:, g % 2, :],
                                    start=g < 4,
                                    stop=g >= NMT - 4,
                                    tile_position=(0, 32 * jj),
                                )
                            if STAGE < 6:
                                continue
                            # fold + approximate reciprocal + normalize + out
                            nc.scalar.copy(out=lrow_sb, in_=lrow)
                            l_rep = outp.tile([128, QC], f32, tag="l_rep")
                            r_rep = outp.tile([128, QC], f32, tag="r_rep")
                            nc.gpsimd.partition_all_reduce(
                                l_rep, lrow_sb, 128, bass_isa.ReduceOp.add
                            )
                            nc.vector.reciprocal_approx_fast(out=r_rep, in_=l_rep)
                            for ci, (av_sb, srcr) in enumerate(
                                ((av0_sb, srcr0), (av1_sb, srcr1))
                            ):
                                o = outp.tile([128, QC], f32, tag=f"o{ci}")
                                nc.vector.tensor_mul(o, av_sb, r_rep)
                                nc.vector.tensor_add(o, o, srcr[:, qc, :])
                                nc.sync.dma_start(
                                    out=out_d[
                                        128 * ci : 128 * (ci + 1),
                                        qc * QC : (qc + 1) * QC,
                                    ],
                                    in_=o,
                                )
    nc.compile()
    return nc


_cached = None


def _get_bass():
    global _cached
    if _cached is None:
        _cached = _build_bass()
    return _cached


def kernel(src_feat, tgt_feat, Wq, bq, Wk, bk, Wv, bv):
    """Full inputs in, full output out. Shards internally across 8 cores."""
    global _last_results
    from concourse.bass_utils import run_bass_kernel_spmd

    src = np.asarray(src_feat, dtype=np.float32).reshape(B, C, N)
    tgt = np.asarray(tgt_feat, dtype=np.float32).reshape(B, C, N)
    # Schraudolph pre-scale folded into the Q projection (scores scale with A)
    wqT = np.ascontiguousarray(np.asarray(Wq, np.float32).T * AEXP).astype(BF16)
    wkT = np.ascontiguousarray(np.asarray(Wk, np.float32).T).astype(BF16)
    wvT = np.ascontiguousarray(np.asarray(Wv, np.float32).T).astype(BF16)
    bqk = np.tile(
        np.stack(
            [np.asarray(bq, np.float32) * AEXP, np.asarray(bk, np.float32)], axis=1
        ),
        (4, 1),
    )  # [128, 2]
    bvr = np.asarray(bv, np.float32).reshape(1, C).astype(BF16)

    tgt_bf = tgt.astype(BF16)

    in_maps = []
    for c in range(NCORES):
        b, h = divmod(c, 2)
        qsl = slice(h * QSH, (h + 1) * QSH)
        in_maps.append(
            {
                "tgt": np.ascontiguousarray(tgt_bf[b]),
                "srcq": np.ascontiguousarray(src[b, :, qsl]).astype(BF16),
                "srcr": np.ascontiguousarray(src[b, :, qsl]),
                "wqT": wqT,
                "wkT": wkT,
                "wvT": wvT,
                "bqk": np.ascontiguousarray(bqk),
                "bv": bvr,
            }
        )

    nc = _get_bass()
    res = None
    for attempt in range(3):
        try:
            res = run_bass_kernel_spmd(
                nc,
                in_maps,
                core_ids=list(range(NCORES)),
                trace=bool(int(os.environ.get("KERNEL_TRACE", "0"))),
            )
            break
        except Exception:
            # the axon-tunneled devices occasionally report
            # NRT_EXEC_UNIT_UNRECOVERABLE; a retry on a fresh execute recovers
            if attempt == 2:
                raise
            import time as _time

            _time.sleep(5)
    _last_results = res

    out = np.empty((B, C, N), dtype=np.float32)
    for c in range(NCORES):
        b, h = divmod(c, 2)
        out[b, :, h * QSH : (h + 1) * QSH] = res.results[c]["out"]
    return out.reshape(B, C, H, W)
